# revision 12
# baseline (speedup 1.0000x reference)
"""Causal self-attention kernel for 8 trn2 NeuronCores.

Sharding: core c = 2*b + g handles batch b (of 4) and head-group g (of 2,
8 heads each).  Each core computes QKV projection, causal attention and the
partial output projection for its head-group; the host sums the two
head-group partials per batch (the w_proj row-split all-reduce done on host).

Matmuls run in bf16 with fp32 PSUM accumulation.  Attention is computed in
transposed orientation (S^T = K Q^T with heads-on-partitions Q/K) so softmax
needs no on-chip transposes; the softmax denominator comes free from a
ones-column appended to V (M=65 PV matmul).

This version streams the QKV projection panel-by-panel interleaved with the
attention panels (flash-style), trims score/PV matmuls to the causal
triangle at 128-column granularity, masks only the 128x128 diagonal blocks,
and normalizes straight out of PSUM.
"""

import sys

if "/opt/trn_rl_repo" not in sys.path:
    sys.path.insert(0, "/opt/trn_rl_repo")

from collections import deque
from contextlib import ExitStack

import numpy as np

import concourse.bass as bass
import concourse.mybir as mybir
import concourse.tile as tile
from concourse import bacc
from concourse.bass_utils import run_bass_kernel_spmd
from concourse.masks import make_identity

F32 = mybir.dt.float32
BF16 = mybir.dt.bfloat16
AF = mybir.ActivationFunctionType

B, T, C = 4, 2048, 1024
N_HEAD = 16
HEAD_DIM = 64
N_CORES = 8
HPC = 8          # heads per core
GC = 512         # head-group channel width (8 heads * 64)
SCALE = 0.125    # 1/sqrt(64)
NP = T // 512    # 512-token panels


def build_program():
    nc = bacc.Bacc(
        "TRN2", target_bir_lowering=False, debug=False, num_devices=N_CORES
    )
    x_ap = nc.dram_tensor("x", [T, C], F32, kind="ExternalInput").ap()
    wq_ap = nc.dram_tensor("wq", [C, GC], F32, kind="ExternalInput").ap()
    wk_ap = nc.dram_tensor("wk", [C, GC], F32, kind="ExternalInput").ap()
    wv_ap = nc.dram_tensor("wv", [C, GC], F32, kind="ExternalInput").ap()
    wp_ap = nc.dram_tensor("wp", [GC, C], F32, kind="ExternalInput").ap()
    out_ap = nc.dram_tensor("out", [T, C], F32, kind="ExternalOutput").ap()

    with ExitStack() as ctx:
        tc = ctx.enter_context(tile.TileContext(nc))
        build_kernel(ctx, tc, x_ap, wq_ap, wk_ap, wv_ap, wp_ap, out_ap)

    nc.compile()
    return nc


def build_kernel(ctx, tc, x_ap, wq_ap, wk_ap, wv_ap, wp_ap, out_ap):
    nc = tc.nc

    # ---------------- constants ----------------
    consts = ctx.enter_context(tc.tile_pool(name="consts", bufs=1))
    ident32 = consts.tile([128, 128], F32)
    make_identity(nc, ident32)
    identb = consts.tile([128, 128], BF16)
    nc.vector.tensor_copy(out=identb, in_=ident32)
    # causal triangle for the diagonal 128x128 blocks:
    # dmask[k, q] = 1 if q >= k else 0
    dmask32 = consts.tile([128, 128], F32)
    nc.gpsimd.memset(dmask32, 1.0)
    nc.gpsimd.affine_select(
        out=dmask32,
        in_=dmask32,
        compare_op=mybir.AluOpType.is_ge,
        fill=0.0,
        base=0,
        pattern=[[1, 128]],
        channel_multiplier=-1,
    )
    dmaskb = consts.tile([128, 128], BF16)
    nc.vector.tensor_copy(out=dmaskb, in_=dmask32)

    # ---------------- persistent QKV outputs ----------------
    qkv_pool = ctx.enter_context(tc.tile_pool(name="qkv", bufs=1))
    QT = [qkv_pool.tile([128, T], BF16, tag=f"qt{i}", name=f"qt{i}") for i in range(4)]
    KT = [qkv_pool.tile([128, T], BF16, tag=f"kt{i}", name=f"kt{i}") for i in range(4)]
    V65 = [
        qkv_pool.tile([128, HPC * 65], BF16, tag=f"v{i}", name=f"v{i}")
        for i in range(16)
    ]
    for i in range(16):
        nc.gpsimd.memset(
            V65[i].rearrange("p (h e) -> p h e", e=65)[:, :, 64:65], 1.0
        )

    # ---------------- weights (loaded once, cast to bf16) ----------------
    wpool = ctx.enter_context(tc.tile_pool(name="w", bufs=1))
    wstage = ctx.enter_context(tc.tile_pool(name="wstage", bufs=1))

    def load_w3(name, ap):
        chunks = []
        for cb in range(8):
            stg = wstage.tile([128, GC], F32, tag="wstg", name="wstg", bufs=4)
            nc.sync.dma_start(out=stg, in_=ap[128 * cb : 128 * cb + 128, :])
            t = wpool.tile([128, GC], BF16, tag=f"{name}{cb}", name=f"{name}{cb}")
            nc.gpsimd.tensor_copy(out=t, in_=stg)
            chunks.append(t)
        return chunks

    def load_wp():
        chunks = []
        for cb in range(4):
            stg = wstage.tile([128, C], F32, tag="wpstg", name="wpstg", bufs=2)
            nc.sync.dma_start(out=stg, in_=wp_ap[128 * cb : 128 * cb + 128, :])
            t = wpool.tile([128, C], BF16, tag=f"wp{cb}", name=f"wpc{cb}")
            nc.gpsimd.tensor_copy(out=t, in_=stg)
            chunks.append(t)
        return chunks

    # ---------------- pools ----------------
    xpool = ctx.enter_context(tc.tile_pool(name="xp", bufs=1))
    xTpool = ctx.enter_context(tc.tile_pool(name="xTp", bufs=2))
    accp = ctx.enter_context(tc.tile_pool(name="accp", bufs=2, space="PSUM"))
    spp = ctx.enter_context(tc.tile_pool(name="spp", bufs=2, space="PSUM"))
    yp = ctx.enter_context(tc.tile_pool(name="yp", bufs=1, space="PSUM"))
    expool = ctx.enter_context(tc.tile_pool(name="exp", bufs=6))
    npool = ctx.enter_context(tc.tile_pool(name="np", bufs=2))
    ytpool = ctx.enter_context(tc.tile_pool(name="ytp", bufs=3))
    otpool = ctx.enter_context(tc.tile_pool(name="otp", bufs=3))

    filler = deque()

    def drain(n):
        for _ in range(n):
            if not filler:
                return
            filler.popleft()()

    def drain_all():
        while filler:
            filler.popleft()()

    # ---------------- QKV projection units ----------------
    def load_x_panel(p):
        """DMA + cast x panel p now; queue its 4 transpose units."""
        t0 = p * 512
        xT = xTpool.tile([128, 8 * 512], BF16, tag="xT", name="xT")
        for ts in range(4):
            xin = xpool.tile([128, C], F32, tag="xin", name="xin", bufs=4)
            nc.sync.dma_start(
                out=xin, in_=x_ap[t0 + 128 * ts : t0 + 128 * ts + 128, :]
            )
            xb = xpool.tile([128, C], BF16, tag="xb", name="xb", bufs=4)
            nc.vector.tensor_copy(out=xb, in_=xin)

            def tr_unit(xb=xb, ts=ts):
                pt = accp.tile([128, 1024], BF16, tag="acc", name="pt")
                for cb in range(8):
                    nc.tensor.transpose(
                        pt[:, 128 * cb : 128 * cb + 128],
                        xb[:, 128 * cb : 128 * cb + 128],
                        identb,
                    )
                src = pt.rearrange("p (cb q) -> p cb q", q=128)
                dst = xT.rearrange("p (cb w) -> p cb w", w=512)[
                    :, :, 128 * ts : 128 * ts + 128
                ]
                nc.vector.tensor_copy(out=dst, in_=src)

            filler.append(tr_unit)
        return xT

    def qkv_units(p, xT):
        qt_u, kt_u, v_u = [], [], []
        for dest, w_sb, lst in ((QT, wq_sb, qt_u), (KT, wk_sb, kt_u)):
            for cp in range(4):

                def unit(cp=cp, dest=dest, w_sb=w_sb):
                    acc = accp.tile([128, 512], F32, tag="acc", name="acc")
                    for cb in range(8):
                        nc.tensor.matmul(
                            acc,
                            w_sb[cb][:, 128 * cp : 128 * cp + 128],
                            xT[:, 512 * cb : 512 * cb + 512],
                            start=(cb == 0),
                            stop=(cb == 7),
                        )
                    nc.vector.tensor_copy(
                        out=dest[cp][:, 512 * p : 512 * p + 512], in_=acc
                    )

                lst.append(unit)
        for ts in range(4):

            def unit(ts=ts):
                acc = accp.tile([128, 512], F32, tag="acc", name="acc")
                for cb in range(8):
                    nc.tensor.matmul(
                        acc,
                        xT[:, 512 * cb + 128 * ts : 512 * cb + 128 * ts + 128],
                        wv_sb[cb],
                        start=(cb == 0),
                        stop=(cb == 7),
                    )
                vt = V65[4 * p + ts]
                nc.vector.tensor_copy(
                    out=vt.rearrange("p (h e) -> p h e", e=65)[:, :, 0:64],
                    in_=acc.rearrange("p (h e) -> p h e", e=64),
                )

            v_u.append(unit)
        return qt_u, kt_u, v_u

    # ---------------- output projection units ----------------
    def proj_units(p, yts):
        units = []
        holder = {}
        for ts in range(4):
            for co in range(2):

                def unit(ts=ts, co=co):
                    if co == 0:
                        holder[ts] = otpool.tile(
                            [128, C], F32, tag="ot", name="ot", bufs=3
                        )
                    ot = holder[ts]
                    ops = accp.tile([128, 512], F32, tag="acc", name="ops")
                    for cp in range(4):
                        nc.tensor.matmul(
                            ops,
                            yts[cp][:, 128 * ts : 128 * ts + 128],
                            wp_sb[cp][:, 512 * co : 512 * co + 512],
                            start=(cp == 0),
                            stop=(cp == 3),
                        )
                    nc.vector.tensor_copy(
                        out=ot[:, 512 * co : 512 * co + 512], in_=ops
                    )
                    if co == 1:
                        nc.sync.dma_start(
                            out=out_ap[
                                512 * p + 128 * ts : 512 * p + 128 * ts + 128, :
                            ],
                            in_=ot,
                        )

                units.append(unit)
        return units

    # ---------------- attention ----------------
    def emit_attention(p):
        q0 = 512 * p
        njd = 2 * (p + 1)
        yts = [
            ytpool.tile([128, 512], BF16, tag=f"yt{i}", name=f"yt{i}", bufs=3)
            for i in range(4)
        ]
        # spread the filler units evenly over the panel's drain slots
        # (skipping the first few so filler never waits on in-flight x DMAs)
        n_slots = 4 * (njd + 1)
        first = 4
        usable = n_slots - first
        n_units = len(filler)
        slot = [0]

        def slot_drain():
            s = slot[0]
            slot[0] += 1
            if s < first or usable <= 0:
                return
            u = s - first
            want = (u + 1) * n_units // usable
            done = n_units - len(filler)
            if want > done:
                drain(want - done)

        for pair in range(4):
            ha, hb = 2 * pair, 2 * pair + 1
            ch = pair
            ypsums = [
                yp.tile([128, 512], F32, tag="ya", name="ya"),
                yp.tile([128, 512], F32, tag="yb", name="yb"),
            ]

            def consume(jd, sps):
                d = jd - 2 * p
                for hi in range(2):
                    h = ha if hi == 0 else hb
                    ex = expool.tile([128, 1024], BF16, tag="ex", name="ex", bufs=6)
                    if d >= 0:
                        qo = (128 * (2 * d), 128 * (2 * d + 1))
                        for half in range(2):
                            o = 512 * half
                            nc.scalar.activation(
                                out=ex[:, o + qo[half] : o + 512],
                                in_=sps[hi][:, o + qo[half] : o + 512],
                                func=AF.Exp,
                                scale=SCALE,
                            )
                        for half in range(2):
                            o = 512 * half
                            nc.vector.tensor_mul(
                                ex[:, o + qo[half] : o + qo[half] + 128],
                                ex[:, o + qo[half] : o + qo[half] + 128],
                                dmaskb,
                            )
                    else:
                        nc.scalar.activation(
                            out=ex, in_=sps[hi], func=AF.Exp, scale=SCALE
                        )
                    for half in range(2):
                        kb = 2 * jd + half
                        i = kb - 4 * p
                        qoff = 128 * i if i >= 0 else 0
                        nc.tensor.matmul(
                            ypsums[hi][0:65, qoff:512],
                            V65[kb][:, 65 * h : 65 * h + 65],
                            ex[:, 512 * half + qoff : 512 * half + 512],
                            start=(jd == 0 and half == 0),
                            stop=(jd == njd - 1 and half == 1),
                        )

            pending = None
            for jd in range(njd):
                sps = [
                    spp.tile([128, 1024], F32, tag="sp", name=f"sp{hi}")
                    for hi in range(2)
                ]
                # halves outer, heads inner: adjacent matmuls hit different
                # PE row groups (base partition 0 vs 64).
                for half in range(2):
                    kb = 2 * jd + half
                    i = kb - 4 * p
                    qoff = 128 * i if i >= 0 else 0
                    for hi in range(2):
                        r0, r1 = (0, 64) if hi == 0 else (64, 128)
                        nc.tensor.matmul(
                            sps[hi][:, 512 * half + qoff : 512 * half + 512],
                            KT[ch][r0:r1, 128 * kb : 128 * kb + 128],
                            QT[ch][r0:r1, q0 + qoff : q0 + 512],
                            start=True,
                            stop=True,
                        )
                slot_drain()
                if pending is not None:
                    consume(*pending)
                pending = (jd, sps)
            consume(*pending)

            # normalize: yt rows = ypsum[0:64] * (1/rowsum) straight from
            # PSUM; rowsum is ypsum row 64 (the V ones-column).
            for hi, h in ((0, ha), (1, hb)):
                rs = npool.tile([1, 512], F32, tag="rs", name="rs", bufs=2)
                nc.vector.tensor_copy(out=rs, in_=ypsums[hi][64:65, :])
                rec = npool.tile([1, 512], F32, tag="rec", name="rec", bufs=2)
                nc.vector.reciprocal_approx_fast(out=rec, in_=rs)
                rb = npool.tile([64, 512], F32, tag="rb", name="rb", bufs=3)
                nc.gpsimd.partition_broadcast(rb, rec)
                r0 = 64 * (h % 2)
                nc.vector.tensor_mul(
                    yts[h // 2][r0 : r0 + 64, :], ypsums[hi][0:64, :], rb
                )
            slot_drain()
        return yts

    # ---------------- schedule ----------------
    xT = load_x_panel(0)
    wq_sb = load_w3("wq", wq_ap)
    drain_all()  # panel-0 transposes
    wk_sb = load_w3("wk", wk_ap)
    wv_sb = load_w3("wv", wv_ap)
    qt_u, kt_u, v_u = qkv_units(0, xT)
    for u in qt_u + kt_u + v_u:
        u()

    yts_hist = {}
    reserve = []
    for p in range(NP):
        if p + 1 < NP:
            xTn = load_x_panel(p + 1)  # queues transpose units
            if p == 1:
                wp_sb = load_wp()
            qt_u, kt_u, v_u = qkv_units(p + 1, xTn)
            filler.extend(qt_u)
            filler.extend(kt_u)
            filler.extend(v_u)
        if p == 2:
            filler.extend(proj_units(0, yts_hist[0]))
        if p == 3:
            filler.extend(proj_units(1, yts_hist[1]))
            pu2 = proj_units(2, yts_hist[2])
            filler.extend(pu2[:4])
            reserve = pu2[4:]  # tail filler: runs while the last norms land
        yts_hist[p] = emit_attention(p)
        drain_all()
    # tail: interleave the held-back proj(2) units with proj(3) so output
    # DMAs start while the last norms land
    p3 = proj_units(3, yts_hist[3])
    tail = []
    while reserve or p3:
        if reserve:
            tail.extend(reserve[:2])
            reserve = reserve[2:]
        tail.extend(p3[:2])
        p3 = p3[2:]
    for u in tail:
        u()


_PROGRAM = None


def _get_program():
    global _PROGRAM
    if _PROGRAM is None:
        _PROGRAM = build_program()
    return _PROGRAM


def make_in_maps(x, w_qkv, w_proj):
    x = np.asarray(x, dtype=np.float32)
    w_qkv = np.asarray(w_qkv, dtype=np.float32)
    w_proj = np.asarray(w_proj, dtype=np.float32)
    in_maps = []
    for core in range(N_CORES):
        b, g = core // 2, core % 2
        c0 = GC * g
        in_maps.append(
            {
                "x": np.ascontiguousarray(x[b]),
                "wq": np.ascontiguousarray(w_qkv[:, c0 : c0 + GC]),
                "wk": np.ascontiguousarray(w_qkv[:, C + c0 : C + c0 + GC]),
                "wv": np.ascontiguousarray(w_qkv[:, 2 * C + c0 : 2 * C + c0 + GC]),
                "wp": np.ascontiguousarray(w_proj[c0 : c0 + GC, :]),
            }
        )
    return in_maps


def combine_outputs(results):
    out = np.empty((B, T, C), dtype=np.float32)
    for b in range(B):
        out[b] = results[2 * b]["out"] + results[2 * b + 1]["out"]
    return out


def kernel(x, w_qkv, w_proj):
    nc = _get_program()
    in_maps = make_in_maps(x, w_qkv, w_proj)
    res = run_bass_kernel_spmd(nc, in_maps, list(range(N_CORES)))
    return combine_outputs(res.results)


if __name__ == "__main__":
    rng = np.random.default_rng(0)
    x = rng.standard_normal((B, T, C), dtype=np.float32)
    wq = rng.standard_normal((C, 3 * C), dtype=np.float32) / 32.0
    wp = rng.standard_normal((C, C), dtype=np.float32) / 32.0
    out = kernel(x, wq, wp)
    print("ok", out.shape, float(np.abs(out).max()))


# revision 13
# speedup vs baseline: 1.1823x; 1.1823x over previous
"""Causal self-attention kernel for 8 trn2 NeuronCores.

Sharding: core c = 2*b + g handles batch b (of 4) and head-group g (of 2,
8 heads each).  Each core computes QKV projection, causal attention and the
partial output projection for its head-group; the host sums the two
head-group partials per batch (the w_proj row-split all-reduce done on host).

Matmuls run in bf16 with fp32 PSUM accumulation.  Attention is computed in
transposed orientation (S^T = K Q^T with heads-on-partitions Q/K) so softmax
needs no on-chip transposes; the softmax denominator comes free from a
ones-column appended to V (M=65 PV matmul).

This version streams the QKV projection panel-by-panel interleaved with the
attention panels (flash-style), trims score/PV matmuls to the causal
triangle at 128-column granularity, masks only the 128x128 diagonal blocks,
and normalizes straight out of PSUM.
"""

import sys

if "/opt/trn_rl_repo" not in sys.path:
    sys.path.insert(0, "/opt/trn_rl_repo")

from collections import deque
from contextlib import ExitStack

import numpy as np

import concourse.bass as bass
import concourse.mybir as mybir
import concourse.tile as tile
from concourse import bacc
from concourse.bass_utils import run_bass_kernel_spmd
from concourse.masks import make_identity

F32 = mybir.dt.float32
BF16 = mybir.dt.bfloat16
AF = mybir.ActivationFunctionType

B, T, C = 4, 2048, 1024
N_HEAD = 16
HEAD_DIM = 64
N_CORES = 8
HPC = 8          # heads per core
GC = 512         # head-group channel width (8 heads * 64)
SCALE = 0.125    # 1/sqrt(64)
NP = T // 512    # 512-token panels


def build_program():
    nc = bacc.Bacc(
        "TRN2", target_bir_lowering=False, debug=False, num_devices=N_CORES
    )
    x_ap = nc.dram_tensor("x", [T, C], F32, kind="ExternalInput").ap()
    wq_ap = nc.dram_tensor("wq", [C, GC], F32, kind="ExternalInput").ap()
    wk_ap = nc.dram_tensor("wk", [C, GC], F32, kind="ExternalInput").ap()
    wv_ap = nc.dram_tensor("wv", [C, GC], F32, kind="ExternalInput").ap()
    wp_ap = nc.dram_tensor("wp", [GC, C], F32, kind="ExternalInput").ap()
    out_ap = nc.dram_tensor("out", [T, C], F32, kind="ExternalOutput").ap()

    with ExitStack() as ctx:
        tc = ctx.enter_context(tile.TileContext(nc))
        build_kernel(ctx, tc, x_ap, wq_ap, wk_ap, wv_ap, wp_ap, out_ap)

    nc.compile()
    return nc


def build_kernel(ctx, tc, x_ap, wq_ap, wk_ap, wv_ap, wp_ap, out_ap):
    nc = tc.nc

    # ---------------- constants ----------------
    consts = ctx.enter_context(tc.tile_pool(name="consts", bufs=1))
    ident32 = consts.tile([128, 128], F32)
    make_identity(nc, ident32)
    identb = consts.tile([128, 128], BF16)
    nc.vector.tensor_copy(out=identb, in_=ident32)
    # causal triangle for the diagonal 128x128 blocks:
    # dmask[k, q] = 1 if q >= k else 0
    dmask32 = consts.tile([128, 128], F32)
    nc.gpsimd.memset(dmask32, 1.0)
    nc.gpsimd.affine_select(
        out=dmask32,
        in_=dmask32,
        compare_op=mybir.AluOpType.is_ge,
        fill=0.0,
        base=0,
        pattern=[[1, 128]],
        channel_multiplier=-1,
    )
    dmaskb = consts.tile([128, 128], BF16)
    nc.vector.tensor_copy(out=dmaskb, in_=dmask32)

    # ---------------- persistent QKV outputs ----------------
    qkv_pool = ctx.enter_context(tc.tile_pool(name="qkv", bufs=1))
    QT = [qkv_pool.tile([128, T], BF16, tag=f"qt{i}", name=f"qt{i}") for i in range(4)]
    KT = [qkv_pool.tile([128, T], BF16, tag=f"kt{i}", name=f"kt{i}") for i in range(4)]
    V65 = [
        qkv_pool.tile([128, HPC * 65], BF16, tag=f"v{i}", name=f"v{i}")
        for i in range(16)
    ]
    for i in range(16):
        nc.gpsimd.memset(
            V65[i].rearrange("p (h e) -> p h e", e=65)[:, :, 64:65], 1.0
        )

    # ---------------- weights (loaded once, cast to bf16) ----------------
    wpool = ctx.enter_context(tc.tile_pool(name="w", bufs=1))
    wstage = ctx.enter_context(tc.tile_pool(name="wstage", bufs=1))

    def load_w3(name, ap):
        chunks = []
        for cb in range(8):
            stg = wstage.tile([128, GC], F32, tag="wstg", name="wstg", bufs=4)
            nc.sync.dma_start(out=stg, in_=ap[128 * cb : 128 * cb + 128, :])
            t = wpool.tile([128, GC], BF16, tag=f"{name}{cb}", name=f"{name}{cb}")
            nc.gpsimd.tensor_copy(out=t, in_=stg)
            chunks.append(t)
        return chunks

    def load_wp():
        chunks = []
        for cb in range(4):
            stg = wstage.tile([128, C], F32, tag="wpstg", name="wpstg", bufs=2)
            nc.sync.dma_start(out=stg, in_=wp_ap[128 * cb : 128 * cb + 128, :])
            t = wpool.tile([128, C], BF16, tag=f"wp{cb}", name=f"wpc{cb}")
            nc.gpsimd.tensor_copy(out=t, in_=stg)
            chunks.append(t)
        return chunks

    # ---------------- pools ----------------
    xpool = ctx.enter_context(tc.tile_pool(name="xp", bufs=1))
    xTpool = ctx.enter_context(tc.tile_pool(name="xTp", bufs=2))
    accp = ctx.enter_context(tc.tile_pool(name="accp", bufs=2, space="PSUM"))
    spp = ctx.enter_context(tc.tile_pool(name="spp", bufs=2, space="PSUM"))
    yp = ctx.enter_context(tc.tile_pool(name="yp", bufs=1, space="PSUM"))
    expool = ctx.enter_context(tc.tile_pool(name="exp", bufs=6))
    npool = ctx.enter_context(tc.tile_pool(name="np", bufs=2))
    ytpool = ctx.enter_context(tc.tile_pool(name="ytp", bufs=3))
    otpool = ctx.enter_context(tc.tile_pool(name="otp", bufs=3))

    filler = deque()

    def drain(n):
        for _ in range(n):
            if not filler:
                return
            filler.popleft()()

    def drain_all():
        while filler:
            filler.popleft()()

    # ---------------- QKV projection units ----------------
    def load_x_panel(p):
        """DMA + cast x panel p now; queue its 4 transpose units."""
        t0 = p * 512
        xT = xTpool.tile([128, 8 * 512], BF16, tag="xT", name="xT")
        for ts in range(4):
            xin = xpool.tile([128, C], F32, tag="xin", name="xin", bufs=4)
            nc.sync.dma_start(
                out=xin, in_=x_ap[t0 + 128 * ts : t0 + 128 * ts + 128, :]
            )
            xb = xpool.tile([128, C], BF16, tag="xb", name="xb", bufs=4)
            nc.scalar.activation(out=xb, in_=xin, func=AF.Copy)

            def tr_unit(xb=xb, ts=ts):
                pt = accp.tile([128, 1024], BF16, tag="acc", name="pt")
                for cb in range(8):
                    nc.tensor.transpose(
                        pt[:, 128 * cb : 128 * cb + 128],
                        xb[:, 128 * cb : 128 * cb + 128],
                        identb,
                    )
                src = pt.rearrange("p (cb q) -> p cb q", q=128)
                dst = xT.rearrange("p (cb w) -> p cb w", w=512)[
                    :, :, 128 * ts : 128 * ts + 128
                ]
                nc.vector.tensor_copy(out=dst, in_=src)

            filler.append(tr_unit)
        return xT

    def qkv_units(p, xT):
        qt_u, kt_u, v_u = [], [], []
        for dest, w_sb, lst in ((QT, wq_sb, qt_u), (KT, wk_sb, kt_u)):
            for cp in range(4):

                def unit(cp=cp, dest=dest, w_sb=w_sb):
                    acc = accp.tile([128, 512], F32, tag="acc", name="acc")
                    for cb in range(8):
                        nc.tensor.matmul(
                            acc,
                            w_sb[cb][:, 128 * cp : 128 * cp + 128],
                            xT[:, 512 * cb : 512 * cb + 512],
                            start=(cb == 0),
                            stop=(cb == 7),
                        )
                    nc.vector.tensor_copy(
                        out=dest[cp][:, 512 * p : 512 * p + 512], in_=acc
                    )

                lst.append(unit)
        for ts in range(4):

            def unit(ts=ts):
                acc = accp.tile([128, 512], F32, tag="acc", name="acc")
                for cb in range(8):
                    nc.tensor.matmul(
                        acc,
                        xT[:, 512 * cb + 128 * ts : 512 * cb + 128 * ts + 128],
                        wv_sb[cb],
                        start=(cb == 0),
                        stop=(cb == 7),
                    )
                vt = V65[4 * p + ts]
                nc.vector.tensor_copy(
                    out=vt.rearrange("p (h e) -> p h e", e=65)[:, :, 0:64],
                    in_=acc.rearrange("p (h e) -> p h e", e=64),
                )

            v_u.append(unit)
        return qt_u, kt_u, v_u

    # ---------------- output projection units ----------------
    def proj_units(p, yts):
        units = []
        holder = {}
        for ts in range(4):
            for co in range(2):

                def unit(ts=ts, co=co):
                    if co == 0:
                        holder[ts] = otpool.tile(
                            [128, C], F32, tag="ot", name="ot", bufs=3
                        )
                    ot = holder[ts]
                    ops = accp.tile([128, 512], F32, tag="acc", name="ops")
                    for cp in range(4):
                        nc.tensor.matmul(
                            ops,
                            yts[cp][:, 128 * ts : 128 * ts + 128],
                            wp_sb[cp][:, 512 * co : 512 * co + 512],
                            start=(cp == 0),
                            stop=(cp == 3),
                        )
                    nc.vector.tensor_copy(
                        out=ot[:, 512 * co : 512 * co + 512], in_=ops
                    )
                    if co == 1:
                        nc.sync.dma_start(
                            out=out_ap[
                                512 * p + 128 * ts : 512 * p + 128 * ts + 128, :
                            ],
                            in_=ot,
                        )

                units.append(unit)
        return units

    # ---------------- attention ----------------
    def emit_attention(p):
        q0 = 512 * p
        njd = 2 * (p + 1)
        yts = [
            ytpool.tile([128, 512], BF16, tag=f"yt{i}", name=f"yt{i}", bufs=3)
            for i in range(4)
        ]
        # spread the filler units evenly over the panel's drain slots
        # (skipping the first few so filler never waits on in-flight x DMAs)
        n_slots = 4 * (njd + 1)
        first = 4
        usable = n_slots - first
        n_units = len(filler)
        slot = [0]

        def slot_drain():
            s = slot[0]
            slot[0] += 1
            if s < first or usable <= 0:
                return
            u = s - first
            want = (u + 1) * n_units // usable
            done = n_units - len(filler)
            if want > done:
                drain(want - done)

        for pair in range(4):
            ha, hb = 2 * pair, 2 * pair + 1
            ch = pair
            ypsums = [
                yp.tile([128, 512], F32, tag="ya", name="ya"),
                yp.tile([128, 512], F32, tag="yb", name="yb"),
            ]

            def consume(jd, sps):
                d = jd - 2 * p
                for hi in range(2):
                    h = ha if hi == 0 else hb
                    ex = expool.tile([128, 1024], BF16, tag="ex", name="ex", bufs=6)
                    if d >= 0:
                        qo = (128 * (2 * d), 128 * (2 * d + 1))
                        for half in range(2):
                            o = 512 * half
                            nc.scalar.activation(
                                out=ex[:, o + qo[half] : o + 512],
                                in_=sps[hi][:, o + qo[half] : o + 512],
                                func=AF.Exp,
                                scale=SCALE,
                            )
                        for half in range(2):
                            o = 512 * half
                            nc.vector.tensor_mul(
                                ex[:, o + qo[half] : o + qo[half] + 128],
                                ex[:, o + qo[half] : o + qo[half] + 128],
                                dmaskb,
                            )
                    else:
                        nc.scalar.activation(
                            out=ex, in_=sps[hi], func=AF.Exp, scale=SCALE
                        )
                    for half in range(2):
                        kb = 2 * jd + half
                        i = kb - 4 * p
                        qoff = 128 * i if i >= 0 else 0
                        nc.tensor.matmul(
                            ypsums[hi][0:65, qoff:512],
                            V65[kb][:, 65 * h : 65 * h + 65],
                            ex[:, 512 * half + qoff : 512 * half + 512],
                            start=(jd == 0 and half == 0),
                            stop=(jd == njd - 1 and half == 1),
                        )

            pending = None
            for jd in range(njd):
                sps = [
                    spp.tile([128, 1024], F32, tag="sp", name=f"sp{hi}")
                    for hi in range(2)
                ]
                # halves outer, heads inner: adjacent matmuls hit different
                # PE row groups (base partition 0 vs 64).
                for half in range(2):
                    kb = 2 * jd + half
                    i = kb - 4 * p
                    qoff = 128 * i if i >= 0 else 0
                    for hi in range(2):
                        r0, r1 = (0, 64) if hi == 0 else (64, 128)
                        nc.tensor.matmul(
                            sps[hi][:, 512 * half + qoff : 512 * half + 512],
                            KT[ch][r0:r1, 128 * kb : 128 * kb + 128],
                            QT[ch][r0:r1, q0 + qoff : q0 + 512],
                            start=True,
                            stop=True,
                        )
                slot_drain()
                if pending is not None:
                    consume(*pending)
                pending = (jd, sps)
            consume(*pending)

            # normalize: yt rows = ypsum[0:64] * (1/rowsum) straight from
            # PSUM; rowsum is ypsum row 64 (the V ones-column).
            for hi, h in ((0, ha), (1, hb)):
                rs = npool.tile([1, 512], F32, tag="rs", name="rs", bufs=2)
                nc.vector.tensor_copy(out=rs, in_=ypsums[hi][64:65, :])
                rec = npool.tile([1, 512], F32, tag="rec", name="rec", bufs=2)
                nc.vector.reciprocal_approx_fast(out=rec, in_=rs)
                rb = npool.tile([64, 512], F32, tag="rb", name="rb", bufs=3)
                nc.gpsimd.partition_broadcast(rb, rec)
                r0 = 64 * (h % 2)
                nc.vector.tensor_mul(
                    yts[h // 2][r0 : r0 + 64, :], ypsums[hi][0:64, :], rb
                )
            slot_drain()
        return yts

    # ---------------- schedule ----------------
    xT = load_x_panel(0)
    wq_sb = load_w3("wq", wq_ap)
    drain_all()  # panel-0 transposes
    wk_sb = load_w3("wk", wk_ap)
    wv_sb = load_w3("wv", wv_ap)
    qt_u, kt_u, v_u = qkv_units(0, xT)
    for u in qt_u + kt_u + v_u:
        u()

    yts_hist = {}
    reserve = []
    for p in range(NP):
        if p + 1 < NP:
            xTn = load_x_panel(p + 1)  # queues transpose units
            if p == 1:
                wp_sb = load_wp()
            qt_u, kt_u, v_u = qkv_units(p + 1, xTn)
            filler.extend(qt_u)
            filler.extend(kt_u)
            filler.extend(v_u)
        if p == 2:
            filler.extend(proj_units(0, yts_hist[0]))
        if p == 3:
            filler.extend(proj_units(1, yts_hist[1]))
            pu2 = proj_units(2, yts_hist[2])
            filler.extend(pu2[:4])
            reserve = pu2[4:]  # tail filler: runs while the last norms land
        yts_hist[p] = emit_attention(p)
        drain_all()
    # tail: interleave the held-back proj(2) units with proj(3) so output
    # DMAs start while the last norms land
    p3 = proj_units(3, yts_hist[3])
    tail = []
    while reserve or p3:
        if reserve:
            tail.extend(reserve[:2])
            reserve = reserve[2:]
        tail.extend(p3[:2])
        p3 = p3[2:]
    for u in tail:
        u()


_PROGRAM = None


def _get_program():
    global _PROGRAM
    if _PROGRAM is None:
        _PROGRAM = build_program()
    return _PROGRAM


def make_in_maps(x, w_qkv, w_proj):
    x = np.asarray(x, dtype=np.float32)
    w_qkv = np.asarray(w_qkv, dtype=np.float32)
    w_proj = np.asarray(w_proj, dtype=np.float32)
    in_maps = []
    for core in range(N_CORES):
        b, g = core // 2, core % 2
        c0 = GC * g
        in_maps.append(
            {
                "x": np.ascontiguousarray(x[b]),
                "wq": np.ascontiguousarray(w_qkv[:, c0 : c0 + GC]),
                "wk": np.ascontiguousarray(w_qkv[:, C + c0 : C + c0 + GC]),
                "wv": np.ascontiguousarray(w_qkv[:, 2 * C + c0 : 2 * C + c0 + GC]),
                "wp": np.ascontiguousarray(w_proj[c0 : c0 + GC, :]),
            }
        )
    return in_maps


def combine_outputs(results):
    out = np.empty((B, T, C), dtype=np.float32)
    for b in range(B):
        out[b] = results[2 * b]["out"] + results[2 * b + 1]["out"]
    return out


def kernel(x, w_qkv, w_proj):
    nc = _get_program()
    in_maps = make_in_maps(x, w_qkv, w_proj)
    res = run_bass_kernel_spmd(nc, in_maps, list(range(N_CORES)))
    return combine_outputs(res.results)


if __name__ == "__main__":
    rng = np.random.default_rng(0)
    x = rng.standard_normal((B, T, C), dtype=np.float32)
    wq = rng.standard_normal((C, 3 * C), dtype=np.float32) / 32.0
    wp = rng.standard_normal((C, C), dtype=np.float32) / 32.0
    out = kernel(x, wq, wp)
    print("ok", out.shape, float(np.abs(out).max()))


# revision 18
# speedup vs baseline: 1.1968x; 1.0122x over previous
"""Causal self-attention kernel for 8 trn2 NeuronCores.

Sharding: core c = 2*b + g handles batch b (of 4) and head-group g (of 2,
8 heads each).  Each core computes QKV projection, causal attention and the
partial output projection for its head-group; the host sums the two
head-group partials per batch (the w_proj row-split all-reduce done on host).

Matmuls run in bf16 with fp32 PSUM accumulation.  Attention is computed in
transposed orientation (S^T = K Q^T with heads-on-partitions Q/K) so softmax
needs no on-chip transposes; the softmax denominator comes free from a
ones-column appended to V (M=65 PV matmul).

This version streams the QKV projection panel-by-panel interleaved with the
attention panels (flash-style), trims score/PV matmuls to the causal
triangle at 128-column granularity, masks only the 128x128 diagonal blocks,
and normalizes straight out of PSUM.
"""

import sys

if "/opt/trn_rl_repo" not in sys.path:
    sys.path.insert(0, "/opt/trn_rl_repo")

from collections import deque
from contextlib import ExitStack

import numpy as np

import concourse.bass as bass
import concourse.mybir as mybir
import concourse.tile as tile
from concourse import bacc
from concourse.bass_utils import run_bass_kernel_spmd
from concourse.masks import make_identity

F32 = mybir.dt.float32
BF16 = mybir.dt.bfloat16
AF = mybir.ActivationFunctionType

B, T, C = 4, 2048, 1024
N_HEAD = 16
HEAD_DIM = 64
N_CORES = 8
HPC = 8          # heads per core
GC = 512         # head-group channel width (8 heads * 64)
SCALE = 0.125    # 1/sqrt(64)
NP = T // 512    # 512-token panels


def build_program():
    nc = bacc.Bacc(
        "TRN2", target_bir_lowering=False, debug=False, num_devices=N_CORES
    )
    x_ap = nc.dram_tensor("x", [T, C], F32, kind="ExternalInput").ap()
    wq_ap = nc.dram_tensor("wq", [C, GC], F32, kind="ExternalInput").ap()
    wk_ap = nc.dram_tensor("wk", [C, GC], F32, kind="ExternalInput").ap()
    wv_ap = nc.dram_tensor("wv", [C, GC], F32, kind="ExternalInput").ap()
    wp_ap = nc.dram_tensor("wp", [GC, C], F32, kind="ExternalInput").ap()
    out_ap = nc.dram_tensor("out", [T, C], F32, kind="ExternalOutput").ap()

    with ExitStack() as ctx:
        tc = ctx.enter_context(tile.TileContext(nc))
        build_kernel(ctx, tc, x_ap, wq_ap, wk_ap, wv_ap, wp_ap, out_ap)

    nc.compile()
    return nc


def build_kernel(ctx, tc, x_ap, wq_ap, wk_ap, wv_ap, wp_ap, out_ap):
    nc = tc.nc

    # ---------------- constants ----------------
    consts = ctx.enter_context(tc.tile_pool(name="consts", bufs=1))
    ident32 = consts.tile([128, 128], F32)
    make_identity(nc, ident32)
    identb = consts.tile([128, 128], BF16)
    nc.vector.tensor_copy(out=identb, in_=ident32)
    # causal triangle for the diagonal 128x128 blocks:
    # dmask[k, q] = 1 if q >= k else 0
    dmask32 = consts.tile([128, 128], F32)
    nc.gpsimd.memset(dmask32, 1.0)
    nc.gpsimd.affine_select(
        out=dmask32,
        in_=dmask32,
        compare_op=mybir.AluOpType.is_ge,
        fill=0.0,
        base=0,
        pattern=[[1, 128]],
        channel_multiplier=-1,
    )
    dmaskb = consts.tile([128, 128], BF16)
    nc.vector.tensor_copy(out=dmaskb, in_=dmask32)

    # ---------------- persistent QKV outputs ----------------
    qkv_pool = ctx.enter_context(tc.tile_pool(name="qkv", bufs=1))
    QT = [qkv_pool.tile([128, T], BF16, tag=f"qt{i}", name=f"qt{i}") for i in range(4)]
    KT = [qkv_pool.tile([128, T], BF16, tag=f"kt{i}", name=f"kt{i}") for i in range(4)]
    V65 = [
        qkv_pool.tile([128, HPC * 65], BF16, tag=f"v{i}", name=f"v{i}")
        for i in range(16)
    ]
    for i in range(16):
        nc.gpsimd.memset(
            V65[i].rearrange("p (h e) -> p h e", e=65)[:, :, 64:65], 1.0
        )

    # ---------------- weights (loaded once, cast to bf16) ----------------
    wpool = ctx.enter_context(tc.tile_pool(name="w", bufs=1))
    wstage = ctx.enter_context(tc.tile_pool(name="wstage", bufs=1))

    def load_w3(name, ap):
        chunks = []
        for cb in range(8):
            stg = wstage.tile([128, GC], F32, tag="wstg", name="wstg", bufs=4)
            nc.sync.dma_start(out=stg, in_=ap[128 * cb : 128 * cb + 128, :])
            t = wpool.tile([128, GC], BF16, tag=f"{name}{cb}", name=f"{name}{cb}")
            nc.gpsimd.tensor_copy(out=t, in_=stg)
            chunks.append(t)
        return chunks

    def load_wp():
        chunks = []
        for cb in range(4):
            stg = wstage.tile([128, C], F32, tag="wpstg", name="wpstg", bufs=2)
            nc.sync.dma_start(out=stg, in_=wp_ap[128 * cb : 128 * cb + 128, :])
            t = wpool.tile([128, C], BF16, tag=f"wp{cb}", name=f"wpc{cb}")
            nc.gpsimd.tensor_copy(out=t, in_=stg)
            chunks.append(t)
        return chunks

    # ---------------- pools ----------------
    xpool = ctx.enter_context(tc.tile_pool(name="xp", bufs=1))
    xTpool = ctx.enter_context(tc.tile_pool(name="xTp", bufs=2))
    accp = ctx.enter_context(tc.tile_pool(name="accp", bufs=2, space="PSUM"))
    spp = ctx.enter_context(tc.tile_pool(name="spp", bufs=2, space="PSUM"))
    yp = ctx.enter_context(tc.tile_pool(name="yp", bufs=1, space="PSUM"))
    expool = ctx.enter_context(tc.tile_pool(name="exp", bufs=6))
    npool = ctx.enter_context(tc.tile_pool(name="np", bufs=2))
    ytpool = ctx.enter_context(tc.tile_pool(name="ytp", bufs=3))
    otpool = ctx.enter_context(tc.tile_pool(name="otp", bufs=3))

    filler = deque()

    def drain(n):
        for _ in range(n):
            if not filler:
                return
            filler.popleft()()

    def drain_all():
        while filler:
            filler.popleft()()

    # ---------------- QKV projection units ----------------
    def load_x_panel(p):
        """DMA + cast x panel p now; queue its 4 transpose units."""
        t0 = p * 512
        xT = xTpool.tile([128, 8 * 512], BF16, tag="xT", name="xT")
        for ts in range(4):
            xin = xpool.tile([128, C], F32, tag="xin", name="xin", bufs=4)
            nc.sync.dma_start(
                out=xin, in_=x_ap[t0 + 128 * ts : t0 + 128 * ts + 128, :]
            )
            xb = xpool.tile([128, C], BF16, tag="xb", name="xb", bufs=4)
            nc.scalar.activation(out=xb, in_=xin, func=AF.Copy)

            def tr_unit(xb=xb, ts=ts):
                pt = accp.tile([128, 1024], BF16, tag="acc", name="pt")
                for cb in range(8):
                    nc.tensor.transpose(
                        pt[:, 128 * cb : 128 * cb + 128],
                        xb[:, 128 * cb : 128 * cb + 128],
                        identb,
                    )
                src = pt.rearrange("p (cb q) -> p cb q", q=128)
                dst = xT.rearrange("p (cb w) -> p cb w", w=512)[
                    :, :, 128 * ts : 128 * ts + 128
                ]
                nc.vector.tensor_copy(out=dst, in_=src)

            filler.append(tr_unit)
        return xT

    def qkv_units(p, xT):
        qt_u, kt_u, v_u = [], [], []
        for dest, w_sb, lst in ((QT, wq_sb, qt_u), (KT, wk_sb, kt_u)):
            for cp in range(4):

                def unit(cp=cp, dest=dest, w_sb=w_sb):
                    acc = accp.tile([128, 512], F32, tag="acc", name="acc")
                    for cb in range(8):
                        nc.tensor.matmul(
                            acc,
                            w_sb[cb][:, 128 * cp : 128 * cp + 128],
                            xT[:, 512 * cb : 512 * cb + 512],
                            start=(cb == 0),
                            stop=(cb == 7),
                        )
                    nc.vector.tensor_copy(
                        out=dest[cp][:, 512 * p : 512 * p + 512], in_=acc
                    )

                lst.append(unit)
        for ts in range(4):

            def unit(ts=ts):
                acc = accp.tile([128, 512], F32, tag="acc", name="acc")
                for cb in range(8):
                    nc.tensor.matmul(
                        acc,
                        xT[:, 512 * cb + 128 * ts : 512 * cb + 128 * ts + 128],
                        wv_sb[cb],
                        start=(cb == 0),
                        stop=(cb == 7),
                    )
                vt = V65[4 * p + ts]
                nc.vector.tensor_copy(
                    out=vt.rearrange("p (h e) -> p h e", e=65)[:, :, 0:64],
                    in_=acc.rearrange("p (h e) -> p h e", e=64),
                )

            v_u.append(unit)
        return qt_u, kt_u, v_u

    # ---------------- output projection units ----------------
    def proj_units(p, yts):
        units = []
        holder = {}
        for ts in range(4):
            for co in range(2):

                def unit(ts=ts, co=co):
                    if co == 0:
                        holder[ts] = otpool.tile(
                            [128, C], F32, tag="ot", name="ot", bufs=3
                        )
                    ot = holder[ts]
                    ops = accp.tile([128, 512], F32, tag="acc", name="ops")
                    for cp in range(4):
                        nc.tensor.matmul(
                            ops,
                            yts[cp][:, 128 * ts : 128 * ts + 128],
                            wp_sb[cp][:, 512 * co : 512 * co + 512],
                            start=(cp == 0),
                            stop=(cp == 3),
                        )
                    nc.vector.tensor_copy(
                        out=ot[:, 512 * co : 512 * co + 512], in_=ops
                    )
                    if co == 1:
                        nc.sync.dma_start(
                            out=out_ap[
                                512 * p + 128 * ts : 512 * p + 128 * ts + 128, :
                            ],
                            in_=ot,
                        )

                units.append(unit)
        return units

    # ---------------- attention ----------------
    def emit_attention(p, late=None, late_units=0):
        q0 = 512 * p
        njd = 2 * (p + 1)
        yts = [
            ytpool.tile([128, 512], BF16, tag=f"yt{i}", name=f"yt{i}", bufs=3)
            for i in range(4)
        ]
        # spread the filler units evenly over the panel's drain slots;
        # `late` queues next-panel loads at the pair-1 boundary so its DMAs
        # and casts never collide with this panel's pipeline priming.
        n_slots = 4 * (njd + 1)
        n_units = len(filler) + late_units
        slot = [0]
        drained = [0]

        def slot_drain():
            s = slot[0]
            slot[0] += 1
            want = (s + 1) * n_units // n_slots
            while drained[0] < want and filler:
                filler.popleft()()
                drained[0] += 1

        for pair in range(4):
            if pair == 1 and late is not None:
                late()
            ha, hb = 2 * pair, 2 * pair + 1
            ch = pair
            ypsums = [
                yp.tile([128, 512], F32, tag="ya", name="ya"),
                yp.tile([128, 512], F32, tag="yb", name="yb"),
            ]

            def consume(jd, sps):
                d = jd - 2 * p
                for hi in range(2):
                    h = ha if hi == 0 else hb
                    ex = expool.tile([128, 1024], BF16, tag="ex", name="ex", bufs=6)
                    if d >= 0:
                        qo = (128 * (2 * d), 128 * (2 * d + 1))
                        for half in range(2):
                            o = 512 * half
                            nc.scalar.activation(
                                out=ex[:, o + qo[half] : o + 512],
                                in_=sps[hi][:, o + qo[half] : o + 512],
                                func=AF.Exp,
                                scale=SCALE,
                            )
                        for half in range(2):
                            o = 512 * half
                            nc.vector.tensor_mul(
                                ex[:, o + qo[half] : o + qo[half] + 128],
                                ex[:, o + qo[half] : o + qo[half] + 128],
                                dmaskb,
                            )
                    else:
                        nc.scalar.activation(
                            out=ex, in_=sps[hi], func=AF.Exp, scale=SCALE
                        )
                    for half in range(2):
                        kb = 2 * jd + half
                        i = kb - 4 * p
                        qoff = 128 * i if i >= 0 else 0
                        nc.tensor.matmul(
                            ypsums[hi][0:65, qoff:512],
                            V65[kb][:, 65 * h : 65 * h + 65],
                            ex[:, 512 * half + qoff : 512 * half + 512],
                            start=(jd == 0 and half == 0),
                            stop=(jd == njd - 1 and half == 1),
                        )

            pending = None
            for jd in range(njd):
                sps = [
                    spp.tile([128, 1024], F32, tag="sp", name=f"sp{hi}")
                    for hi in range(2)
                ]
                # halves outer, heads inner: adjacent matmuls hit different
                # PE row groups (base partition 0 vs 64).
                for half in range(2):
                    kb = 2 * jd + half
                    i = kb - 4 * p
                    qoff = 128 * i if i >= 0 else 0
                    for hi in range(2):
                        r0, r1 = (0, 64) if hi == 0 else (64, 128)
                        nc.tensor.matmul(
                            sps[hi][:, 512 * half + qoff : 512 * half + 512],
                            KT[ch][r0:r1, 128 * kb : 128 * kb + 128],
                            QT[ch][r0:r1, q0 + qoff : q0 + 512],
                            start=True,
                            stop=True,
                        )
                slot_drain()
                if pending is not None:
                    consume(*pending)
                pending = (jd, sps)
            consume(*pending)

            # normalize: yt rows = ypsum[0:64] * (1/rowsum) straight from
            # PSUM; rowsum is ypsum row 64 (the V ones-column).
            for hi, h in ((0, ha), (1, hb)):
                rs = npool.tile([1, 512], F32, tag="rs", name="rs", bufs=2)
                nc.vector.tensor_copy(out=rs, in_=ypsums[hi][64:65, :])
                rec = npool.tile([1, 512], F32, tag="rec", name="rec", bufs=2)
                nc.vector.reciprocal_approx_fast(out=rec, in_=rs)
                rb = npool.tile([64, 512], F32, tag="rb", name="rb", bufs=3)
                nc.gpsimd.partition_broadcast(rb, rec)
                r0 = 64 * (h % 2)
                nc.vector.tensor_mul(
                    yts[h // 2][r0 : r0 + 64, :], ypsums[hi][0:64, :], rb
                )
            slot_drain()
        return yts

    # ---------------- schedule ----------------
    wp_sb = None
    xT = load_x_panel(0)
    wq_sb = load_w3("wq", wq_ap)
    drain_all()  # panel-0 transposes
    wk_sb = load_w3("wk", wk_ap)
    wv_sb = load_w3("wv", wv_ap)
    qt_u, kt_u, v_u = qkv_units(0, xT)
    for u in qt_u + kt_u + v_u:
        u()

    yts_hist = {}
    reserve = []
    for p in range(NP):
        late = None
        late_units = 0
        if p + 1 < NP:

            def late(p=p):
                nonlocal wp_sb
                xTn = load_x_panel(p + 1)  # queues 4 transpose units
                if p == 1:
                    wp_sb = load_wp()
                qt_u, kt_u, v_u = qkv_units(p + 1, xTn)
                filler.extend(qt_u)
                filler.extend(kt_u)
                filler.extend(v_u)

            late_units = 16
        if p == 2:
            filler.extend(proj_units(0, yts_hist[0]))
        if p == 3:
            filler.extend(proj_units(1, yts_hist[1]))
            pu2 = proj_units(2, yts_hist[2])
            filler.extend(pu2[:4])
            reserve = pu2[4:]  # tail filler: runs while the last norms land
        yts_hist[p] = emit_attention(p, late, late_units)
        drain_all()
    for u in reserve:
        u()
    for u in proj_units(3, yts_hist[3]):
        u()


_PROGRAM = None


def _get_program():
    global _PROGRAM
    if _PROGRAM is None:
        _PROGRAM = build_program()
    return _PROGRAM


def make_in_maps(x, w_qkv, w_proj):
    x = np.asarray(x, dtype=np.float32)
    w_qkv = np.asarray(w_qkv, dtype=np.float32)
    w_proj = np.asarray(w_proj, dtype=np.float32)
    in_maps = []
    for core in range(N_CORES):
        b, g = core // 2, core % 2
        c0 = GC * g
        in_maps.append(
            {
                "x": np.ascontiguousarray(x[b]),
                "wq": np.ascontiguousarray(w_qkv[:, c0 : c0 + GC]),
                "wk": np.ascontiguousarray(w_qkv[:, C + c0 : C + c0 + GC]),
                "wv": np.ascontiguousarray(w_qkv[:, 2 * C + c0 : 2 * C + c0 + GC]),
                "wp": np.ascontiguousarray(w_proj[c0 : c0 + GC, :]),
            }
        )
    return in_maps


def combine_outputs(results):
    out = np.empty((B, T, C), dtype=np.float32)
    for b in range(B):
        out[b] = results[2 * b]["out"] + results[2 * b + 1]["out"]
    return out


def kernel(x, w_qkv, w_proj):
    nc = _get_program()
    in_maps = make_in_maps(x, w_qkv, w_proj)
    res = run_bass_kernel_spmd(nc, in_maps, list(range(N_CORES)))
    return combine_outputs(res.results)


if __name__ == "__main__":
    rng = np.random.default_rng(0)
    x = rng.standard_normal((B, T, C), dtype=np.float32)
    wq = rng.standard_normal((C, 3 * C), dtype=np.float32) / 32.0
    wp = rng.standard_normal((C, C), dtype=np.float32) / 32.0
    out = kernel(x, wq, wp)
    print("ok", out.shape, float(np.abs(out).max()))
